# revision 4
# baseline (speedup 1.0000x reference)
"""Trainium2 Bass kernel for CompositionalMHA (moe_routing).

Math (see reference):
  For each bank b in {q,k,v}:  proj_b = sum_{j in top4(softmax(logits_b))}
      tw_j * (x @ U_j @ V_j)
  Then 16-head causal attention over the projections, then out @ out_w.T.

Host side: the top-k selection + softmax weights depend only on the tiny
logits vectors, so they are computed here in numpy; the selected U banks are
concatenated into [d, 4*64] and the tw-scaled V banks into [4*64, d_out].

Sharding (8 cores): core c = (batch b = c//2, head-half g = c%2).
Each core gets x[b] (transposed to [d,S]), the full U-cat per bank, the
head-half columns of V-cat per bank, and the matching 512 rows of out_w.T.
It computes a partial [S, d_model] output (its 8 heads' contribution through
the output projection); the host sums the two half-contributions per batch.

Device kernel works entirely in "transposed activation" layout [feat, S]:
  hT = Ucat^T @ xT           (contract d)
  qT/kT = Vw^T @ hT          (contract 4*64)    -> [512, S]
  v    = hT^T @ Vw           (per s-tile)       -> [S, 512] (natural layout)
  scoresT[k,q] = k_h @ q_h^T per head           -> exp -> causal mask-mul
  outT[65, q]  = [v_h | 1]^T @ probsT           (row 64 = softmax denom)
  attnT = outT[0:64] * (1/denom broadcast via rank-1 matmul)
  final[s, m] = attnT^T @ w_half                (contract feature)
Softmax skips max-subtraction: scores*scale for these inputs are O(1), far
from fp32 exp overflow, and softmax normalization is scale-invariant.
"""

import numpy as np

import concourse.bass as bass
import concourse.bacc as bacc
import concourse.mybir as mybir
import concourse.tile as tile
from concourse.bass_utils import run_bass_kernel_spmd

F32 = mybir.dt.float32
AF = mybir.ActivationFunctionType

P = 128
S = 1024        # sequence length
DM = 1024       # d_model
KR = 256        # top_k * r = 4 * 64
F = 512         # features per core = 8 heads * 64
NH = 8          # heads per core
HD = 64         # head dim
NG_D = DM // P  # 8
NG_R = KR // P  # 2
NG_F = F // P   # 4
NST = S // P    # 8
NSC = S // 512  # 2
MB = 896        # staircase mask width (512 + 3*128)

TRACE = False
_cache = {}


def _emit(nc, tc, xT, us, vs, w, mask, out):
    from contextlib import ExitStack

    with ExitStack() as ctx:
        pp = ctx.enter_context(tc.tile_pool(name="persist", bufs=1))

        xT_sb = pp.tile([P, NG_D, S], F32)
        for g in range(NG_D):
            nc.sync.dma_start(out=xT_sb[:, g, :], in_=xT[g * P:(g + 1) * P, :])
        mask_sb = pp.tile([P, MB], F32)
        nc.sync.dma_start(out=mask_sb, in_=mask)
        w_sb = pp.tile([P, NG_F, DM], F32)
        for g in range(NG_F):
            nc.sync.dma_start(out=w_sb[:, g, :], in_=w[g * P:(g + 1) * P, :])
        u_sb = {}
        vw_sb = {}
        for b in "qkv":
            u_sb[b] = pp.tile([P, NG_D, KR], F32, name=f"u{b}_sb")
            for g in range(NG_D):
                nc.sync.dma_start(out=u_sb[b][:, g, :], in_=us[b][g * P:(g + 1) * P, :])
            vw_sb[b] = pp.tile([P, NG_R, F], F32, name=f"vw{b}_sb")
            for g in range(NG_R):
                nc.sync.dma_start(out=vw_sb[b][:, g, :], in_=vs[b][g * P:(g + 1) * P, :])

        qT_sb = pp.tile([P, NG_F, S], F32)
        kT_sb = pp.tile([P, NG_F, S], F32)
        vS_sb = pp.tile([P, NST, NH, HD + 1], F32)
        nc.vector.memset(vS_sb[:, :, :, HD:HD + 1], 1.0)
        attnT_sb = pp.tile([P, NG_F, S], F32)

        # ---- Phase B: projections ----
        hpool = ctx.enter_context(tc.tile_pool(name="hpool", bufs=2))
        with tc.tile_pool(name="pph", bufs=4, space="PSUM") as pph:
            for b in "qkv":
                hT_sb = hpool.tile([P, NG_R, S], F32, name=f"hT_{b}", tag="hT")
                for mi in range(NG_R):
                    for sc in range(NSC):
                        h_ps = pph.tile([P, 512], F32, name="h_ps", tag="h_ps")
                        for g in range(NG_D):
                            nc.tensor.matmul(
                                h_ps,
                                lhsT=u_sb[b][:, g, mi * P:(mi + 1) * P],
                                rhs=xT_sb[:, g, sc * 512:(sc + 1) * 512],
                                start=(g == 0), stop=(g == NG_D - 1),
                            )
                        nc.vector.tensor_copy(
                            hT_sb[:, mi, sc * 512:(sc + 1) * 512], h_ps)
                if b in "qk":
                    dst = qT_sb if b == "q" else kT_sb
                    for fc in range(NG_F):
                        for sc in range(NSC):
                            b_ps = pph.tile([P, 512], F32, name="b_ps", tag="h_ps")
                            for mi in range(NG_R):
                                nc.tensor.matmul(
                                    b_ps,
                                    lhsT=vw_sb[b][:, mi, fc * P:(fc + 1) * P],
                                    rhs=hT_sb[:, mi, sc * 512:(sc + 1) * 512],
                                    start=(mi == 0), stop=(mi == NG_R - 1),
                                )
                            nc.vector.tensor_copy(
                                dst[:, fc, sc * 512:(sc + 1) * 512], b_ps)
                else:
                    for st in range(NST):
                        v_ps = pph.tile([P, F], F32, name="v_ps", tag="h_ps")
                        for mi in range(NG_R):
                            nc.tensor.matmul(
                                v_ps,
                                lhsT=hT_sb[:, mi, st * P:(st + 1) * P],
                                rhs=vw_sb[b][:, mi, :],
                                start=(mi == 0), stop=(mi == NG_R - 1),
                            )
                        nc.vector.tensor_copy(
                            vS_sb[:, st, :, 0:HD],
                            v_ps.rearrange("p (h e) -> p h e", h=NH))

        # ---- Phase C: attention ----
        spp = ctx.enter_context(tc.tile_pool(name="spp", bufs=4))
        spr = ctx.enter_context(tc.tile_pool(name="spr", bufs=4))
        with (
            tc.tile_pool(name="pps", bufs=4, space="PSUM") as pps,
            tc.tile_pool(name="ppo", bufs=4, space="PSUM") as ppo,
        ):
            for hp in range(NH // 2):
                for qc in range(NSC):
                    n_kt = 4 * (qc + 1)
                    o_ps = [
                        ppo.tile([HD + 1, 512], F32, name=f"o_ps{sub}", tag="o_ps")
                        for sub in range(2)
                    ]
                    for kt in range(n_kt):
                        for sub in range(2):
                            h = 2 * hp + sub
                            po = HD * sub
                            s_ps = pps.tile([P, 512], F32, name="s_ps", tag="s_ps")
                            nc.tensor.matmul(
                                s_ps,
                                lhsT=kT_sb[po:po + HD, hp, kt * P:(kt + 1) * P],
                                rhs=qT_sb[po:po + HD, hp, qc * 512:(qc + 1) * 512],
                                start=True, stop=True,
                            )
                            pT = spp.tile([P, 512], F32, name="pT", tag="pT")
                            nc.scalar.activation(
                                out=pT, in_=s_ps, func=AF.Exp, scale=0.125)
                            rel = P * kt - 512 * qc
                            if rel >= 0:
                                o = 384 - rel
                                nc.vector.tensor_mul(pT, pT, mask_sb[:, o:o + 512])
                            nc.tensor.matmul(
                                o_ps[sub],
                                lhsT=vS_sb[:, kt, h, :],
                                rhs=pT,
                                start=(kt == 0), stop=(kt == n_kt - 1),
                            )
                    for sub in range(2):
                        po = HD * sub
                        rcp = spr.tile([1, 512], F32, name="rcp", tag="rcp")
                        nc.vector.reciprocal(rcp, o_ps[sub][HD:HD + 1, :])
                        bc_sb = spr.tile([HD, 512], F32, name="bc_sb", tag="bc_sb")
                        nc.gpsimd.partition_broadcast(bc_sb, rcp)
                        nc.vector.tensor_mul(
                            attnT_sb[po:po + HD, hp, qc * 512:(qc + 1) * 512],
                            o_ps[sub][0:HD, :], bc_sb)

        # ---- Phase D: output projection ----
        spo = ctx.enter_context(tc.tile_pool(name="spo", bufs=3))
        with tc.tile_pool(name="ppf", bufs=4, space="PSUM") as ppf:
            for st in range(NST):
                for mc in range(NSC):
                    f_ps = ppf.tile([P, 512], F32, name="f_ps", tag="f_ps")
                    for fcc in range(NG_F):
                        nc.tensor.matmul(
                            f_ps,
                            lhsT=attnT_sb[:, fcc, st * P:(st + 1) * P],
                            rhs=w_sb[:, fcc, mc * 512:(mc + 1) * 512],
                            start=(fcc == 0), stop=(fcc == NG_F - 1),
                        )
                    o_sb = spo.tile([P, 512], F32, name="o_sb", tag="o_sb")
                    nc.scalar.copy(out=o_sb, in_=f_ps)
                    nc.sync.dma_start(
                        out=out[st * P:(st + 1) * P, mc * 512:(mc + 1) * 512],
                        in_=o_sb)


def _build():
    nc = bacc.Bacc("TRN2", target_bir_lowering=False, debug=False, num_devices=8)
    xT = nc.dram_tensor("xT", [DM, S], F32, kind="ExternalInput").ap()
    us = {b: nc.dram_tensor(f"u{b}", [DM, KR], F32, kind="ExternalInput").ap()
          for b in "qkv"}
    vs = {b: nc.dram_tensor(f"v{b}", [KR, F], F32, kind="ExternalInput").ap()
          for b in "qkv"}
    w = nc.dram_tensor("w", [F, DM], F32, kind="ExternalInput").ap()
    mask = nc.dram_tensor("mask", [P, MB], F32, kind="ExternalInput").ap()
    out = nc.dram_tensor("out", [S, DM], F32, kind="ExternalOutput").ap()
    with tile.TileContext(nc) as tc:
        _emit(nc, tc, xT, us, vs, w, mask, out)
    nc.compile()
    return nc


def _staircase_mask():
    # mask[rk, j] = 1.0 iff j >= rk + 384; slice [:, 384-rel : 384-rel+512]
    # gives the causal keep-mask for a k-tile at relative offset rel.
    rk = np.arange(P)[:, None]
    j = np.arange(MB)[None, :]
    return (j >= rk + 384).astype(np.float32)


def _select_bank(U, V, logits, top_k):
    lg = np.asarray(logits, np.float32)
    e = np.exp(lg - lg.max())
    wsoft = (e / e.sum()).astype(np.float32)
    ti = np.argsort(-wsoft, kind="stable")[:top_k]
    tw = wsoft[ti]
    tw = tw / tw.sum()
    Ucat = np.concatenate([U[i] for i in ti], axis=1)          # [d, k*r]
    Vcat = np.concatenate([tw[k] * V[ti[k]] for k in range(top_k)], axis=0)
    return np.ascontiguousarray(Ucat, np.float32), np.ascontiguousarray(Vcat, np.float32)


def kernel(**inputs):
    x = np.asarray(inputs["x"], np.float32)          # [4, S, d]
    out_w = np.asarray(inputs["out_w"], np.float32)  # [d, d]
    top_k = int(np.asarray(inputs["top_k"]))
    assert top_k * 64 == KR, f"kernel compiled for top_k=4, got {top_k}"
    B = x.shape[0]

    cats = {}
    for b in "qkv":
        cats[b] = _select_bank(
            np.asarray(inputs[f"{b}_U"], np.float32),
            np.asarray(inputs[f"{b}_V"], np.float32),
            inputs[f"{b}_logits"], top_k)

    if "nc" not in _cache:
        _cache["nc"] = _build()
    nc = _cache["nc"]

    mask = _staircase_mask()
    wT = np.ascontiguousarray(out_w.T, np.float32)   # [feat, d_model]
    in_maps = []
    for c in range(8):
        b, g = c // 2, c % 2
        m = {"xT": np.ascontiguousarray(x[b].T),
             "mask": mask,
             "w": np.ascontiguousarray(wT[g * F:(g + 1) * F, :])}
        for bank in "qkv":
            Ucat, Vcat = cats[bank]
            m[f"u{bank}"] = Ucat
            m[f"v{bank}"] = np.ascontiguousarray(Vcat[:, g * F:(g + 1) * F])
        in_maps.append(m)

    res = run_bass_kernel_spmd(nc, in_maps, core_ids=list(range(8)), trace=TRACE)
    if TRACE:
        _cache["last_results"] = res
    parts = [r["out"] for r in res.results]
    full = np.stack([parts[2 * b] + parts[2 * b + 1] for b in range(B)])
    return full.astype(np.float32)


# revision 13
# speedup vs baseline: 1.8333x; 1.8333x over previous
"""Trainium2 Bass kernel for CompositionalMHA (moe_routing).

Math (see reference):
  For each bank b in {q,k,v}:  proj_b = sum_{j in top4(softmax(logits_b))}
      tw_j * (x @ U_j @ V_j)
  Then 16-head causal attention over the projections, then out @ out_w.T.

Host side: the top-k selection + softmax weights depend only on the tiny
logits vectors, so they are computed here in numpy; the selected U banks are
concatenated into [d, 4*64] and the tw-scaled V banks into [4*64, d_out].

Sharding (8 cores): core c = (batch b = c//2, head-half g = c%2).
Each core gets x[b] (transposed to [d,S]), the full U-cat per bank, the
head-half columns of V-cat per bank, and the matching 512 rows of out_w.T.
It computes a partial [S, d_model] output (its 8 heads' contribution through
the output projection); the host sums the two half-contributions per batch.

Device kernel works entirely in "transposed activation" layout [feat, S]:
  hT = Ucat^T @ xT           (contract d)
  qT/kT = Vw^T @ hT          (contract 4*64)    -> [512, S]
  v    = hT^T @ Vw           (per s-tile)       -> [S, 512] (natural layout)
  scoresT[k,q] = k_h @ q_h^T per head           -> exp -> causal mask
  outT[65, q]  = [v_h | 1]^T @ probsT           (row 64 = softmax denom)
  attnT = outT[0:64] * (1/denom broadcast across partitions)
  final[s, m] = attnT^T @ w_half                (contract feature)

Matmul operands are bitcast to float32r (single-pass PE mode, 1 cycle/row
at N>=512 vs 4 for float32). Softmax skips max-subtraction: scores*scale
for these inputs are O(1), far from fp32 exp overflow, and softmax
normalization is scale-invariant.
"""

import numpy as np

import concourse.bass as bass
import concourse.bacc as bacc
import concourse.mybir as mybir
import concourse.tile as tile
from concourse.bass_utils import run_bass_kernel_spmd

F32 = mybir.dt.float32
F32R = mybir.dt.float32r
AF = mybir.ActivationFunctionType

P = 128
S = 1024        # sequence length
DM = 1024       # d_model
KR = 256        # top_k * r = 4 * 64
F = 512         # features per core = 8 heads * 64
NH = 8          # heads per core
HD = 64         # head dim
NG_D = DM // P  # 8
NG_R = KR // P  # 2
NG_F = F // P   # 4
NST = S // P    # 8
NSC = S // 512  # 2

TRACE = False
_cache = {}


def _mm(nc, out, lhsT, rhs, **kw):
    nc.tensor.matmul(out, lhsT=lhsT.bitcast(F32R), rhs=rhs.bitcast(F32R), **kw)


def _emit(nc, tc, xT, us, vs, w, mask, out):
    from contextlib import ExitStack

    with ExitStack() as ctx:
        pp = ctx.enter_context(tc.tile_pool(name="persist", bufs=1))

        xT_sb = pp.tile([P, NG_D, S], F32R)
        for g in range(NG_D):
            nc.sync.dma_start(out=xT_sb[:, g, :], in_=xT[g * P:(g + 1) * P, :])
        mask_sb = pp.tile([P, P], F32)
        nc.sync.dma_start(out=mask_sb, in_=mask)
        w_sb = pp.tile([P, NG_F, DM], F32R)
        for g in range(NG_F):
            nc.sync.dma_start(out=w_sb[:, g, :], in_=w[g * P:(g + 1) * P, :])

        u_sb = {}
        vw_sb = {}
        for b in "qkv":
            u_sb[b] = pp.tile([P, NG_D, KR], F32R, name=f"u{b}_sb")
            for g in range(NG_D):
                nc.sync.dma_start(out=u_sb[b][:, g, :], in_=us[b][g * P:(g + 1) * P, :])
            vw_sb[b] = pp.tile([P, NG_R, F], F32R, name=f"vw{b}_sb")
            for g in range(NG_R):
                nc.sync.dma_start(out=vw_sb[b][:, g, :], in_=vs[b][g * P:(g + 1) * P, :])

        qT_sb = pp.tile([P, NG_F, S], F32R)
        kT_sb = pp.tile([P, NG_F, S], F32R)
        vS_sb = pp.tile([P, NST, NH, HD + 1], F32R)
        nc.vector.memset(vS_sb[:, :, :, HD:HD + 1].bitcast(F32), 1.0)
        attnT_sb = pp.tile([P, NG_F, S], F32R)
        # softmax denominators: row di lives at partition 32*(di%4),
        # free slot di//4 (ACT output base-partition must be 32-aligned)
        den_sb = pp.tile([P, 4, 512], F32)
        nc.vector.memset(den_sb, 1.0)
        den_dram = nc.dram_tensor("den_scratch", [16, 512], F32,
                                  kind="Internal").ap()

        # ---- Phase B: projections ----
        hpool = ctx.enter_context(tc.tile_pool(name="hpool", bufs=2))
        with tc.tile_pool(name="pph", bufs=4, space="PSUM") as pph:
            for b in "qkv":
                hT_sb = hpool.tile([P, NG_R, S], F32R, name=f"hT_{b}", tag="hT")
                for mi in range(NG_R):
                    for sc in range(NSC):
                        h_ps = pph.tile([P, 512], F32, name="h_ps", tag="h_ps")
                        for g in range(NG_D):
                            _mm(nc, h_ps,
                                u_sb[b][:, g, mi * P:(mi + 1) * P],
                                xT_sb[:, g, sc * 512:(sc + 1) * 512],
                                start=(g == 0), stop=(g == NG_D - 1))
                        nc.vector.tensor_copy(
                            hT_sb[:, mi, sc * 512:(sc + 1) * 512], h_ps)
                if b in "qk":
                    dst = qT_sb if b == "q" else kT_sb
                    for fc in range(NG_F):
                        for sc in range(NSC):
                            b_ps = pph.tile([P, 512], F32, name="b_ps", tag="h_ps")
                            for mi in range(NG_R):
                                _mm(nc, b_ps,
                                    vw_sb[b][:, mi, fc * P:(fc + 1) * P],
                                    hT_sb[:, mi, sc * 512:(sc + 1) * 512],
                                    start=(mi == 0), stop=(mi == NG_R - 1))
                            nc.vector.tensor_copy(
                                dst[:, fc, sc * 512:(sc + 1) * 512], b_ps)
                else:
                    for st in range(NST):
                        v_ps = pph.tile([P, F], F32, name="v_ps", tag="h_ps")
                        for mi in range(NG_R):
                            _mm(nc, v_ps,
                                hT_sb[:, mi, st * P:(st + 1) * P],
                                vw_sb[b][:, mi, :],
                                start=(mi == 0), stop=(mi == NG_R - 1))
                        nc.vector.tensor_copy(
                            vS_sb[:, st, :, 0:HD],
                            v_ps.rearrange("p (h e) -> p h e", h=NH))

        # ---- Phase C: attention ----
        spp = ctx.enter_context(tc.tile_pool(name="spp", bufs=4))
        spr = ctx.enter_context(tc.tile_pool(name="spr", bufs=4))
        with (
            tc.tile_pool(name="pps", bufs=4, space="PSUM") as pps,
            tc.tile_pool(name="ppo", bufs=3, space="PSUM") as ppo,
        ):
            def norm_group(rows):
                # rows: list of (den_row, h, hp, qc)
                s0 = min(r[0] for r in rows) // 4
                s1 = max(r[0] for r in rows) // 4 + 1
                rcp = spr.tile([P, 2, 512], F32, name="rcp", tag="rcp",
                               bufs=2)
                nc.vector.reciprocal(rcp, den_sb[:, s0:s1, :])
                for (di, h, hp, qc) in rows:
                    po = HD * (h % 2)
                    dp, ds_ = 32 * (di % 4), di // 4
                    bc_sb = spr.tile([P, 512], F32, name="bc_sb", tag="bc_sb")
                    # HW partition_broadcast ignores AP offsets, so bounce the
                    # reciprocal row through DRAM and broadcast-load it
                    # (stride-0 partition APs are legal for DRAM sources).
                    nc.sync.dma_start(
                        out=den_dram[di:di + 1, :],
                        in_=rcp[dp:dp + 1, ds_ - s0, :])
                    nc.sync.dma_start(
                        out=bc_sb,
                        in_=bass.AP(
                            tensor=den_dram.tensor,
                            offset=den_dram[di:di + 1, :].offset,
                            ap=[[0, P], [1, 512]]))
                    sl = attnT_sb[po:po + HD, hp, qc * 512:(qc + 1) * 512]
                    nc.vector.tensor_mul(sl, sl, bc_sb[po:po + HD, :])

            pend = []
            for hp in range(NH // 2):
                for qc in range(NSC):
                    n_kt = 4 * (qc + 1)
                    o_ps = [
                        ppo.tile([HD + 1, 512], F32, name=f"o_ps{sub}", tag="o_ps")
                        for sub in range(2)
                    ]
                    for kt in range(n_kt):
                        rel = P * kt - 512 * qc
                        for sub in range(2):
                            h = 2 * hp + sub
                            po = HD * sub
                            s_ps = pps.tile([P, 512], F32, name="s_ps", tag="s_ps")
                            _mm(nc, s_ps,
                                kT_sb[po:po + HD, hp, kt * P:(kt + 1) * P],
                                qT_sb[po:po + HD, hp, qc * 512:(qc + 1) * 512],
                                start=True, stop=True)
                            pT = spp.tile([P, 512], F32R, name="pT", tag="pT")
                            if rel >= 0:
                                # causal-crossing tile: cols < rel are fully
                                # masked, cols [rel, rel+128) need the
                                # triangular mask, cols >= rel+128 are valid.
                                if rel > 0:
                                    nc.gpsimd.memset(pT[:, 0:rel].bitcast(F32), 0.0)
                                nc.scalar.activation(
                                    out=pT[:, rel:512], in_=s_ps[:, rel:512],
                                    func=AF.Exp, scale=0.125)
                                nc.vector.tensor_mul(
                                    pT[:, rel:rel + P], pT[:, rel:rel + P],
                                    mask_sb)
                            else:
                                nc.scalar.activation(
                                    out=pT, in_=s_ps, func=AF.Exp, scale=0.125)
                            _mm(nc, o_ps[sub],
                                vS_sb[:, kt, h, :], pT,
                                start=(kt == 0), stop=(kt == n_kt - 1))
                    for sub in range(2):
                        h = 2 * hp + sub
                        po = HD * sub
                        di = (hp * 2 + qc) * 2 + sub
                        nc.vector.tensor_copy(
                            attnT_sb[po:po + HD, hp, qc * 512:(qc + 1) * 512],
                            o_ps[sub][0:HD, :])
                        nc.scalar.copy(
                            out=den_sb[32 * (di % 4):32 * (di % 4) + 1, di // 4, :],
                            in_=o_ps[sub][HD:HD + 1, :])
                        pend.append((di, h, hp, qc))
                if hp % 2 == 1:
                    norm_group(pend)
                    pend = []

        # ---- Phase D: output projection ----
        spo = ctx.enter_context(tc.tile_pool(name="spo", bufs=3))
        with tc.tile_pool(name="ppf", bufs=4, space="PSUM") as ppf:
            for st in range(NST):
                for mc in range(NSC):
                    f_ps = ppf.tile([P, 512], F32, name="f_ps", tag="f_ps")
                    for fcc in range(NG_F):
                        _mm(nc, f_ps,
                            attnT_sb[:, fcc, st * P:(st + 1) * P],
                            w_sb[:, fcc, mc * 512:(mc + 1) * 512],
                            start=(fcc == 0), stop=(fcc == NG_F - 1))
                    o_sb = spo.tile([P, 512], F32, name="o_sb", tag="o_sb")
                    nc.scalar.copy(out=o_sb, in_=f_ps)
                    nc.sync.dma_start(
                        out=out[st * P:(st + 1) * P, mc * 512:(mc + 1) * 512],
                        in_=o_sb)


def _build():
    nc = bacc.Bacc("TRN2", target_bir_lowering=False, debug=False, num_devices=8)
    xT = nc.dram_tensor("xT", [DM, S], F32R, kind="ExternalInput").ap()
    us = {b: nc.dram_tensor(f"u{b}", [DM, KR], F32R, kind="ExternalInput").ap()
          for b in "qkv"}
    vs = {b: nc.dram_tensor(f"v{b}", [KR, F], F32R, kind="ExternalInput").ap()
          for b in "qkv"}
    w = nc.dram_tensor("w", [F, DM], F32R, kind="ExternalInput").ap()
    mask = nc.dram_tensor("mask", [P, P], F32, kind="ExternalInput").ap()
    out = nc.dram_tensor("out", [S, DM], F32, kind="ExternalOutput").ap()
    with tile.TileContext(nc) as tc:
        _emit(nc, tc, xT, us, vs, w, mask, out)
    nc.compile()
    return nc


def _tri_mask():
    # tri[rk, c] = 1.0 iff c >= rk  (keep where key index <= query index
    # within a diagonal 128x128 block)
    rk = np.arange(P)[:, None]
    c = np.arange(P)[None, :]
    return (c >= rk).astype(np.float32)


def _select_bank(U, V, logits, top_k):
    lg = np.asarray(logits, np.float32)
    e = np.exp(lg - lg.max())
    wsoft = (e / e.sum()).astype(np.float32)
    ti = np.argsort(-wsoft, kind="stable")[:top_k]
    tw = wsoft[ti]
    tw = tw / tw.sum()
    Ucat = np.concatenate([U[i] for i in ti], axis=1)          # [d, k*r]
    Vcat = np.concatenate([tw[k] * V[ti[k]] for k in range(top_k)], axis=0)
    return np.ascontiguousarray(Ucat, np.float32), np.ascontiguousarray(Vcat, np.float32)


def kernel(**inputs):
    x = np.asarray(inputs["x"], np.float32)          # [4, S, d]
    out_w = np.asarray(inputs["out_w"], np.float32)  # [d, d]
    top_k = int(np.asarray(inputs["top_k"]))
    assert top_k * 64 == KR, f"kernel compiled for top_k=4, got {top_k}"
    B = x.shape[0]

    cats = {}
    for b in "qkv":
        cats[b] = _select_bank(
            np.asarray(inputs[f"{b}_U"], np.float32),
            np.asarray(inputs[f"{b}_V"], np.float32),
            inputs[f"{b}_logits"], top_k)

    if "nc" not in _cache:
        _cache["nc"] = _build()
    nc = _cache["nc"]

    mask = _tri_mask()
    wT = np.ascontiguousarray(out_w.T, np.float32)   # [feat, d_model]
    in_maps = []
    for c in range(8):
        b, g = c // 2, c % 2
        m = {"xT": np.ascontiguousarray(x[b].T),
             "mask": mask,
             "w": np.ascontiguousarray(wT[g * F:(g + 1) * F, :])}
        for bank in "qkv":
            Ucat, Vcat = cats[bank]
            m[f"u{bank}"] = Ucat
            m[f"v{bank}"] = np.ascontiguousarray(Vcat[:, g * F:(g + 1) * F])
        in_maps.append(m)

    res = run_bass_kernel_spmd(nc, in_maps, core_ids=list(range(8)), trace=TRACE)
    if TRACE:
        _cache["last_results"] = res
    parts = [r["out"] for r in res.results]
    full = np.stack([parts[2 * b] + parts[2 * b + 1] for b in range(B)])
    return full.astype(np.float32)


# revision 14
# speedup vs baseline: 1.9735x; 1.0765x over previous
"""Trainium2 Bass kernel for CompositionalMHA (moe_routing).

Math (see reference):
  For each bank b in {q,k,v}:  proj_b = sum_{j in top4(softmax(logits_b))}
      tw_j * (x @ U_j @ V_j)
  Then 16-head causal attention over the projections, then out @ out_w.T.

Host side: the top-k selection + softmax weights depend only on the tiny
logits vectors, so they are computed here in numpy; the selected U banks are
concatenated into [d, 4*64] and the tw-scaled V banks into [4*64, d_out].

Sharding (8 cores): core c = (batch b = c//2, head-half g = c%2).
Each core gets x[b] (transposed to [d,S]), the full U-cat per bank, the
head-half columns of V-cat per bank, and the matching 512 rows of out_w.T.
It computes a partial [S, d_model] output (its 8 heads' contribution through
the output projection); the host sums the two half-contributions per batch.

Device kernel works entirely in "transposed activation" layout [feat, S]:
  hT = Ucat^T @ xT           (contract d)
  qT/kT = Vw^T @ hT          (contract 4*64)    -> [512, S]
  v    = hT^T @ Vw           (per s-tile)       -> [S, 512] (natural layout)
  scoresT[k,q] = k_h @ q_h^T per head           -> exp -> causal mask
  outT[65, q]  = [v_h | 1]^T @ probsT           (row 64 = softmax denom)
  attnT = outT[0:64] * (1/denom broadcast across partitions)
  final[s, m] = attnT^T @ w_half                (contract feature)

Matmul operands are bitcast to float32r (single-pass PE mode, 1 cycle/row
at N>=512 vs 4 for float32). Softmax skips max-subtraction: scores*scale
for these inputs are O(1), far from fp32 exp overflow, and softmax
normalization is scale-invariant.
"""

import numpy as np

import concourse.bass as bass
import concourse.bacc as bacc
import concourse.mybir as mybir
import concourse.tile as tile
from concourse.bass_utils import run_bass_kernel_spmd

F32 = mybir.dt.float32
F32R = mybir.dt.float32r
AF = mybir.ActivationFunctionType

P = 128
S = 1024        # sequence length
DM = 1024       # d_model
KR = 256        # top_k * r = 4 * 64
F = 512         # features per core = 8 heads * 64
NH = 8          # heads per core
HD = 64         # head dim
NG_D = DM // P  # 8
NG_R = KR // P  # 2
NG_F = F // P   # 4
NST = S // P    # 8
NSC = S // 512  # 2

TRACE = False
_cache = {}


def _mm(nc, out, lhsT, rhs, **kw):
    nc.tensor.matmul(out, lhsT=lhsT.bitcast(F32R), rhs=rhs.bitcast(F32R), **kw)


def _emit(nc, tc, xT, us, vs, w, mask, out):
    from contextlib import ExitStack

    with ExitStack() as ctx:
        pp = ctx.enter_context(tc.tile_pool(name="persist", bufs=1))

        xT_sb = pp.tile([P, NG_D, S], F32R)
        for g in range(NG_D):
            nc.sync.dma_start(out=xT_sb[:, g, :], in_=xT[g * P:(g + 1) * P, :])
        mask_sb = pp.tile([P, P], F32)
        nc.sync.dma_start(out=mask_sb, in_=mask)
        w_sb = pp.tile([P, NG_F, DM], F32R)
        for g in range(NG_F):
            nc.sync.dma_start(out=w_sb[:, g, :], in_=w[g * P:(g + 1) * P, :])

        u_sb = {}
        vw_sb = {}
        for b in "qkv":
            u_sb[b] = pp.tile([P, NG_D, KR], F32R, name=f"u{b}_sb")
            for g in range(NG_D):
                nc.sync.dma_start(out=u_sb[b][:, g, :], in_=us[b][g * P:(g + 1) * P, :])
            vw_sb[b] = pp.tile([P, NG_R, F], F32R, name=f"vw{b}_sb")
            for g in range(NG_R):
                nc.sync.dma_start(out=vw_sb[b][:, g, :], in_=vs[b][g * P:(g + 1) * P, :])

        qT_sb = pp.tile([P, NG_F, S], F32R)
        kT_sb = pp.tile([P, NG_F, S], F32R)
        vS_sb = pp.tile([P, NST, NH, HD + 1], F32R)
        nc.vector.memset(vS_sb[:, :, :, HD:HD + 1].bitcast(F32), 1.0)
        attnT_sb = pp.tile([P, NG_F, S], F32R)
        # softmax denominators: row di lives at partition 32*(di%4),
        # free slot di//4 (ACT output base-partition must be 32-aligned)
        den_sb = pp.tile([P, 4, 512], F32)
        nc.vector.memset(den_sb, 1.0)
        den_dram = nc.dram_tensor("den_scratch", [16, 512], F32,
                                  kind="Internal").ap()

        # ---- Phase B: projections ----
        hpool = ctx.enter_context(tc.tile_pool(name="hpool", bufs=2))
        with tc.tile_pool(name="pph", bufs=8, space="PSUM") as pph:
            for b in "qkv":
                hT_sb = hpool.tile([P, NG_R, S], F32R, name=f"hT_{b}", tag="hT")
                for mi in range(NG_R):
                    for sc in range(NSC):
                        h_ps = pph.tile([P, 512], F32, name="h_ps", tag="h_ps")
                        for g in range(NG_D):
                            _mm(nc, h_ps,
                                u_sb[b][:, g, mi * P:(mi + 1) * P],
                                xT_sb[:, g, sc * 512:(sc + 1) * 512],
                                start=(g == 0), stop=(g == NG_D - 1))
                        nc.vector.tensor_copy(
                            hT_sb[:, mi, sc * 512:(sc + 1) * 512], h_ps)
                if b in "qk":
                    dst = qT_sb if b == "q" else kT_sb
                    for fc in range(NG_F):
                        for sc in range(NSC):
                            b_ps = pph.tile([P, 512], F32, name="b_ps", tag="h_ps")
                            for mi in range(NG_R):
                                _mm(nc, b_ps,
                                    vw_sb[b][:, mi, fc * P:(fc + 1) * P],
                                    hT_sb[:, mi, sc * 512:(sc + 1) * 512],
                                    start=(mi == 0), stop=(mi == NG_R - 1))
                            nc.vector.tensor_copy(
                                dst[:, fc, sc * 512:(sc + 1) * 512], b_ps)
                else:
                    for st in range(NST):
                        v_ps = pph.tile([P, F], F32, name="v_ps", tag="h_ps")
                        for mi in range(NG_R):
                            _mm(nc, v_ps,
                                hT_sb[:, mi, st * P:(st + 1) * P],
                                vw_sb[b][:, mi, :],
                                start=(mi == 0), stop=(mi == NG_R - 1))
                        nc.vector.tensor_copy(
                            vS_sb[:, st, :, 0:HD],
                            v_ps.rearrange("p (h e) -> p h e", h=NH))

        # ---- Phase C: attention ----
        spp = ctx.enter_context(tc.tile_pool(name="spp", bufs=6))
        spr = ctx.enter_context(tc.tile_pool(name="spr", bufs=4))
        with (
            tc.tile_pool(name="pps", bufs=5, space="PSUM") as pps,
            tc.tile_pool(name="ppo", bufs=3, space="PSUM") as ppo,
        ):
            def norm_group(rows):
                # rows: list of (den_row, h, hp, qc)
                s0 = min(r[0] for r in rows) // 4
                s1 = max(r[0] for r in rows) // 4 + 1
                rcp = spr.tile([P, 2, 512], F32, name="rcp", tag="rcp",
                               bufs=2)
                nc.vector.reciprocal(rcp, den_sb[:, s0:s1, :])
                for (di, h, hp, qc) in rows:
                    po = HD * (h % 2)
                    dp, ds_ = 32 * (di % 4), di // 4
                    bc_sb = spr.tile([P, 512], F32, name="bc_sb", tag="bc_sb")
                    # HW partition_broadcast ignores AP offsets, so bounce the
                    # reciprocal row through DRAM and broadcast-load it
                    # (stride-0 partition APs are legal for DRAM sources).
                    nc.sync.dma_start(
                        out=den_dram[di:di + 1, :],
                        in_=rcp[dp:dp + 1, ds_ - s0, :])
                    nc.sync.dma_start(
                        out=bc_sb,
                        in_=bass.AP(
                            tensor=den_dram.tensor,
                            offset=den_dram[di:di + 1, :].offset,
                            ap=[[0, P], [1, 512]]))
                    sl = attnT_sb[po:po + HD, hp, qc * 512:(qc + 1) * 512]
                    nc.vector.tensor_mul(sl, sl, bc_sb[po:po + HD, :])

            pend = []
            for hp in range(NH // 2):
                for qc in range(NSC):
                    n_kt = 4 * (qc + 1)
                    o_ps = [
                        ppo.tile([HD + 1, 512], F32, name=f"o_ps{sub}", tag="o_ps")
                        for sub in range(2)
                    ]
                    for kt in range(n_kt):
                        rel = P * kt - 512 * qc
                        for sub in range(2):
                            h = 2 * hp + sub
                            po = HD * sub
                            s_ps = pps.tile([P, 512], F32, name="s_ps", tag="s_ps")
                            _mm(nc, s_ps,
                                kT_sb[po:po + HD, hp, kt * P:(kt + 1) * P],
                                qT_sb[po:po + HD, hp, qc * 512:(qc + 1) * 512],
                                start=True, stop=True)
                            pT = spp.tile([P, 512], F32R, name="pT", tag="pT")
                            if rel >= 0:
                                # causal-crossing tile: cols < rel are fully
                                # masked, cols [rel, rel+128) need the
                                # triangular mask, cols >= rel+128 are valid.
                                if rel > 0:
                                    nc.gpsimd.memset(pT[:, 0:rel].bitcast(F32), 0.0)
                                nc.scalar.activation(
                                    out=pT[:, rel:512], in_=s_ps[:, rel:512],
                                    func=AF.Exp, scale=0.125)
                                nc.vector.tensor_mul(
                                    pT[:, rel:rel + P], pT[:, rel:rel + P],
                                    mask_sb)
                            else:
                                nc.scalar.activation(
                                    out=pT, in_=s_ps, func=AF.Exp, scale=0.125)
                            _mm(nc, o_ps[sub],
                                vS_sb[:, kt, h, :], pT,
                                start=(kt == 0), stop=(kt == n_kt - 1))
                    for sub in range(2):
                        h = 2 * hp + sub
                        po = HD * sub
                        di = (hp * 2 + qc) * 2 + sub
                        nc.vector.tensor_copy(
                            attnT_sb[po:po + HD, hp, qc * 512:(qc + 1) * 512],
                            o_ps[sub][0:HD, :])
                        nc.scalar.copy(
                            out=den_sb[32 * (di % 4):32 * (di % 4) + 1, di // 4, :],
                            in_=o_ps[sub][HD:HD + 1, :])
                        pend.append((di, h, hp, qc))
                if hp % 2 == 1:
                    norm_group(pend)
                    pend = []

        # ---- Phase D: output projection ----
        spo = ctx.enter_context(tc.tile_pool(name="spo", bufs=3))
        with tc.tile_pool(name="ppf", bufs=8, space="PSUM") as ppf:
            for st in range(NST):
                for mc in range(NSC):
                    f_ps = ppf.tile([P, 512], F32, name="f_ps", tag="f_ps")
                    for fcc in range(NG_F):
                        _mm(nc, f_ps,
                            attnT_sb[:, fcc, st * P:(st + 1) * P],
                            w_sb[:, fcc, mc * 512:(mc + 1) * 512],
                            start=(fcc == 0), stop=(fcc == NG_F - 1))
                    o_sb = spo.tile([P, 512], F32, name="o_sb", tag="o_sb")
                    nc.scalar.copy(out=o_sb, in_=f_ps)
                    nc.sync.dma_start(
                        out=out[st * P:(st + 1) * P, mc * 512:(mc + 1) * 512],
                        in_=o_sb)


def _build():
    nc = bacc.Bacc("TRN2", target_bir_lowering=False, debug=False, num_devices=8)
    xT = nc.dram_tensor("xT", [DM, S], F32R, kind="ExternalInput").ap()
    us = {b: nc.dram_tensor(f"u{b}", [DM, KR], F32R, kind="ExternalInput").ap()
          for b in "qkv"}
    vs = {b: nc.dram_tensor(f"v{b}", [KR, F], F32R, kind="ExternalInput").ap()
          for b in "qkv"}
    w = nc.dram_tensor("w", [F, DM], F32R, kind="ExternalInput").ap()
    mask = nc.dram_tensor("mask", [P, P], F32, kind="ExternalInput").ap()
    out = nc.dram_tensor("out", [S, DM], F32, kind="ExternalOutput").ap()
    with tile.TileContext(nc) as tc:
        _emit(nc, tc, xT, us, vs, w, mask, out)
    nc.compile()
    return nc


def _tri_mask():
    # tri[rk, c] = 1.0 iff c >= rk  (keep where key index <= query index
    # within a diagonal 128x128 block)
    rk = np.arange(P)[:, None]
    c = np.arange(P)[None, :]
    return (c >= rk).astype(np.float32)


def _select_bank(U, V, logits, top_k):
    lg = np.asarray(logits, np.float32)
    e = np.exp(lg - lg.max())
    wsoft = (e / e.sum()).astype(np.float32)
    ti = np.argsort(-wsoft, kind="stable")[:top_k]
    tw = wsoft[ti]
    tw = tw / tw.sum()
    Ucat = np.concatenate([U[i] for i in ti], axis=1)          # [d, k*r]
    Vcat = np.concatenate([tw[k] * V[ti[k]] for k in range(top_k)], axis=0)
    return np.ascontiguousarray(Ucat, np.float32), np.ascontiguousarray(Vcat, np.float32)


def kernel(**inputs):
    x = np.asarray(inputs["x"], np.float32)          # [4, S, d]
    out_w = np.asarray(inputs["out_w"], np.float32)  # [d, d]
    top_k = int(np.asarray(inputs["top_k"]))
    assert top_k * 64 == KR, f"kernel compiled for top_k=4, got {top_k}"
    B = x.shape[0]

    cats = {}
    for b in "qkv":
        cats[b] = _select_bank(
            np.asarray(inputs[f"{b}_U"], np.float32),
            np.asarray(inputs[f"{b}_V"], np.float32),
            inputs[f"{b}_logits"], top_k)

    if "nc" not in _cache:
        _cache["nc"] = _build()
    nc = _cache["nc"]

    mask = _tri_mask()
    wT = np.ascontiguousarray(out_w.T, np.float32)   # [feat, d_model]
    in_maps = []
    for c in range(8):
        b, g = c // 2, c % 2
        m = {"xT": np.ascontiguousarray(x[b].T),
             "mask": mask,
             "w": np.ascontiguousarray(wT[g * F:(g + 1) * F, :])}
        for bank in "qkv":
            Ucat, Vcat = cats[bank]
            m[f"u{bank}"] = Ucat
            m[f"v{bank}"] = np.ascontiguousarray(Vcat[:, g * F:(g + 1) * F])
        in_maps.append(m)

    res = run_bass_kernel_spmd(nc, in_maps, core_ids=list(range(8)), trace=TRACE)
    if TRACE:
        _cache["last_results"] = res
    parts = [r["out"] for r in res.results]
    full = np.stack([parts[2 * b] + parts[2 * b + 1] for b in range(B)])
    return full.astype(np.float32)


# revision 15
# speedup vs baseline: 2.0602x; 1.0439x over previous
"""Trainium2 Bass kernel for CompositionalMHA (moe_routing).

Math (see reference):
  For each bank b in {q,k,v}:  proj_b = sum_{j in top4(softmax(logits_b))}
      tw_j * (x @ U_j @ V_j)
  Then 16-head causal attention over the projections, then out @ out_w.T.

Host side: the top-k selection + softmax weights depend only on the tiny
logits vectors, so they are computed here in numpy; the selected U banks are
concatenated into [d, 4*64] and the tw-scaled V banks into [4*64, d_out].

Sharding (8 cores): core c = (batch b = c//2, head-half g = c%2).
Each core gets x[b] (transposed to [d,S]), the full U-cat per bank, the
head-half columns of V-cat per bank, and the matching 512 rows of out_w.T.
It computes a partial [S, d_model] output (its 8 heads' contribution through
the output projection); the host sums the two half-contributions per batch.

Device kernel works entirely in "transposed activation" layout [feat, S]:
  hT = Ucat^T @ xT           (contract d)
  qT/kT = Vw^T @ hT          (contract 4*64)    -> [512, S]
  v    = hT^T @ Vw           (per s-tile)       -> [S, 512] (natural layout)
  scoresT[k,q] = k_h @ q_h^T per head           -> exp -> causal mask
  outT[65, q]  = [v_h | 1]^T @ probsT           (row 64 = softmax denom)
  attnT = outT[0:64] * (1/denom broadcast across partitions)
  final[s, m] = attnT^T @ w_half                (contract feature)

Matmul operands are bitcast to float32r (single-pass PE mode, 1 cycle/row
at N>=512 vs 4 for float32). Softmax skips max-subtraction: scores*scale
for these inputs are O(1), far from fp32 exp overflow, and softmax
normalization is scale-invariant.
"""

import ml_dtypes
import numpy as np

import concourse.bass as bass
import concourse.bacc as bacc
import concourse.mybir as mybir
import concourse.tile as tile
from concourse.bass_utils import run_bass_kernel_spmd

F32 = mybir.dt.float32
F32R = mybir.dt.float32r
BF16 = mybir.dt.bfloat16
AF = mybir.ActivationFunctionType

P = 128
S = 1024        # sequence length
DM = 1024       # d_model
KR = 256        # top_k * r = 4 * 64
F = 512         # features per core = 8 heads * 64
NH = 8          # heads per core
HD = 64         # head dim
NG_D = DM // P  # 8
NG_R = KR // P  # 2
NG_F = F // P   # 4
NST = S // P    # 8
NSC = S // 512  # 2

TRACE = False
_cache = {}


def _r(ap):
    return ap.bitcast(F32R) if ap.dtype == F32 else ap


def _mm(nc, out, lhsT, rhs, **kw):
    nc.tensor.matmul(out, lhsT=_r(lhsT), rhs=_r(rhs), **kw)


def _emit(nc, tc, xT, us, vs, w, mask, out):
    from contextlib import ExitStack

    with ExitStack() as ctx:
        pp = ctx.enter_context(tc.tile_pool(name="persist", bufs=1))

        xT_sb = pp.tile([P, NG_D, S], BF16)
        for g in range(NG_D):
            nc.sync.dma_start(out=xT_sb[:, g, :], in_=xT[g * P:(g + 1) * P, :])
        mask_sb = pp.tile([P, P], F32)
        nc.sync.dma_start(out=mask_sb, in_=mask)
        w_sb = pp.tile([P, NG_F, DM], F32R)
        for g in range(NG_F):
            nc.sync.dma_start(out=w_sb[:, g, :], in_=w[g * P:(g + 1) * P, :])

        u_sb = {}
        vw_sb = {}
        for b in "qkv":
            u_sb[b] = pp.tile([P, NG_D, KR], BF16, name=f"u{b}_sb")
            for g in range(NG_D):
                nc.sync.dma_start(out=u_sb[b][:, g, :], in_=us[b][g * P:(g + 1) * P, :])
            vw_sb[b] = pp.tile([P, NG_R, F], BF16, name=f"vw{b}_sb")
            for g in range(NG_R):
                nc.sync.dma_start(out=vw_sb[b][:, g, :], in_=vs[b][g * P:(g + 1) * P, :])

        qT_sb = pp.tile([P, NG_F, S], F32R)
        kT_sb = pp.tile([P, NG_F, S], F32R)
        vS_sb = pp.tile([P, NST, NH, HD + 1], F32R)
        nc.vector.memset(vS_sb[:, :, :, HD:HD + 1].bitcast(F32), 1.0)
        attnT_sb = pp.tile([P, NG_F, S], F32R)
        # softmax denominators: row di lives at partition 32*(di%4),
        # free slot di//4 (ACT output base-partition must be 32-aligned)
        den_sb = pp.tile([P, 4, 512], F32)
        nc.vector.memset(den_sb, 1.0)
        den_dram = nc.dram_tensor("den_scratch", [16, 512], F32,
                                  kind="Internal").ap()

        # ---- Phase B: projections ----
        hpool = ctx.enter_context(tc.tile_pool(name="hpool", bufs=2))
        with tc.tile_pool(name="pph", bufs=8, space="PSUM") as pph:
            for b in "qkv":
                hT_sb = hpool.tile([P, NG_R, S], BF16, name=f"hT_{b}", tag="hT")
                for mi in range(NG_R):
                    for sc in range(NSC):
                        h_ps = pph.tile([P, 512], F32, name="h_ps", tag="h_ps")
                        for g in range(NG_D):
                            _mm(nc, h_ps,
                                u_sb[b][:, g, mi * P:(mi + 1) * P],
                                xT_sb[:, g, sc * 512:(sc + 1) * 512],
                                start=(g == 0), stop=(g == NG_D - 1))
                        nc.vector.tensor_copy(
                            hT_sb[:, mi, sc * 512:(sc + 1) * 512], h_ps)
                if b in "qk":
                    dst = qT_sb if b == "q" else kT_sb
                    for fc in range(NG_F):
                        for sc in range(NSC):
                            b_ps = pph.tile([P, 512], F32, name="b_ps", tag="h_ps")
                            for mi in range(NG_R):
                                _mm(nc, b_ps,
                                    vw_sb[b][:, mi, fc * P:(fc + 1) * P],
                                    hT_sb[:, mi, sc * 512:(sc + 1) * 512],
                                    start=(mi == 0), stop=(mi == NG_R - 1))
                            nc.vector.tensor_copy(
                                dst[:, fc, sc * 512:(sc + 1) * 512], b_ps)
                else:
                    for st in range(NST):
                        v_ps = pph.tile([P, F], F32, name="v_ps", tag="h_ps")
                        for mi in range(NG_R):
                            _mm(nc, v_ps,
                                hT_sb[:, mi, st * P:(st + 1) * P],
                                vw_sb[b][:, mi, :],
                                start=(mi == 0), stop=(mi == NG_R - 1))
                        nc.vector.tensor_copy(
                            vS_sb[:, st, :, 0:HD],
                            v_ps.rearrange("p (h e) -> p h e", h=NH))

        # ---- Phase C: attention ----
        spp = ctx.enter_context(tc.tile_pool(name="spp", bufs=6))
        spr = ctx.enter_context(tc.tile_pool(name="spr", bufs=4))
        with (
            tc.tile_pool(name="pps", bufs=5, space="PSUM") as pps,
            tc.tile_pool(name="ppo", bufs=3, space="PSUM") as ppo,
        ):
            def norm_group(rows):
                # rows: list of (den_row, h, hp, qc)
                s0 = min(r[0] for r in rows) // 4
                s1 = max(r[0] for r in rows) // 4 + 1
                rcp = spr.tile([P, 2, 512], F32, name="rcp", tag="rcp",
                               bufs=2)
                nc.vector.reciprocal(rcp, den_sb[:, s0:s1, :])
                for (di, h, hp, qc) in rows:
                    po = HD * (h % 2)
                    dp, ds_ = 32 * (di % 4), di // 4
                    bc_sb = spr.tile([P, 512], F32, name="bc_sb", tag="bc_sb")
                    # HW partition_broadcast ignores AP offsets, so bounce the
                    # reciprocal row through DRAM and broadcast-load it
                    # (stride-0 partition APs are legal for DRAM sources).
                    nc.sync.dma_start(
                        out=den_dram[di:di + 1, :],
                        in_=rcp[dp:dp + 1, ds_ - s0, :])
                    nc.sync.dma_start(
                        out=bc_sb,
                        in_=bass.AP(
                            tensor=den_dram.tensor,
                            offset=den_dram[di:di + 1, :].offset,
                            ap=[[0, P], [1, 512]]))
                    sl = attnT_sb[po:po + HD, hp, qc * 512:(qc + 1) * 512]
                    nc.vector.tensor_mul(sl, sl, bc_sb[po:po + HD, :])

            pend = []
            for hp in range(NH // 2):
                for qc in range(NSC):
                    n_kt = 4 * (qc + 1)
                    o_ps = [
                        ppo.tile([HD + 1, 512], F32, name=f"o_ps{sub}", tag="o_ps")
                        for sub in range(2)
                    ]
                    for kt in range(n_kt):
                        rel = P * kt - 512 * qc
                        for sub in range(2):
                            h = 2 * hp + sub
                            po = HD * sub
                            s_ps = pps.tile([P, 512], F32, name="s_ps", tag="s_ps")
                            _mm(nc, s_ps,
                                kT_sb[po:po + HD, hp, kt * P:(kt + 1) * P],
                                qT_sb[po:po + HD, hp, qc * 512:(qc + 1) * 512],
                                start=True, stop=True)
                            pT = spp.tile([P, 512], F32R, name="pT", tag="pT")
                            if rel >= 0:
                                # causal-crossing tile: cols < rel are fully
                                # masked, cols [rel, rel+128) need the
                                # triangular mask, cols >= rel+128 are valid.
                                if rel > 0:
                                    nc.gpsimd.memset(pT[:, 0:rel].bitcast(F32), 0.0)
                                nc.scalar.activation(
                                    out=pT[:, rel:512], in_=s_ps[:, rel:512],
                                    func=AF.Exp, scale=0.125)
                                nc.vector.tensor_mul(
                                    pT[:, rel:rel + P], pT[:, rel:rel + P],
                                    mask_sb)
                            else:
                                nc.scalar.activation(
                                    out=pT, in_=s_ps, func=AF.Exp, scale=0.125)
                            _mm(nc, o_ps[sub],
                                vS_sb[:, kt, h, :], pT,
                                start=(kt == 0), stop=(kt == n_kt - 1))
                    for sub in range(2):
                        h = 2 * hp + sub
                        po = HD * sub
                        di = (hp * 2 + qc) * 2 + sub
                        nc.vector.tensor_copy(
                            attnT_sb[po:po + HD, hp, qc * 512:(qc + 1) * 512],
                            o_ps[sub][0:HD, :])
                        nc.scalar.copy(
                            out=den_sb[32 * (di % 4):32 * (di % 4) + 1, di // 4, :],
                            in_=o_ps[sub][HD:HD + 1, :])
                        pend.append((di, h, hp, qc))
                if hp % 2 == 1:
                    norm_group(pend)
                    pend = []

        # ---- Phase D: output projection ----
        spo = ctx.enter_context(tc.tile_pool(name="spo", bufs=3))
        with tc.tile_pool(name="ppf", bufs=8, space="PSUM") as ppf:
            for st in range(NST):
                for mc in range(NSC):
                    f_ps = ppf.tile([P, 512], F32, name="f_ps", tag="f_ps")
                    for fcc in range(NG_F):
                        _mm(nc, f_ps,
                            attnT_sb[:, fcc, st * P:(st + 1) * P],
                            w_sb[:, fcc, mc * 512:(mc + 1) * 512],
                            start=(fcc == 0), stop=(fcc == NG_F - 1))
                    o_sb = spo.tile([P, 512], F32, name="o_sb", tag="o_sb")
                    nc.scalar.copy(out=o_sb, in_=f_ps)
                    nc.sync.dma_start(
                        out=out[st * P:(st + 1) * P, mc * 512:(mc + 1) * 512],
                        in_=o_sb)


def _build():
    nc = bacc.Bacc("TRN2", target_bir_lowering=False, debug=False, num_devices=8)
    xT = nc.dram_tensor("xT", [DM, S], BF16, kind="ExternalInput").ap()
    us = {b: nc.dram_tensor(f"u{b}", [DM, KR], BF16, kind="ExternalInput").ap()
          for b in "qkv"}
    vs = {b: nc.dram_tensor(f"v{b}", [KR, F], BF16, kind="ExternalInput").ap()
          for b in "qkv"}
    w = nc.dram_tensor("w", [F, DM], F32R, kind="ExternalInput").ap()
    mask = nc.dram_tensor("mask", [P, P], F32, kind="ExternalInput").ap()
    out = nc.dram_tensor("out", [S, DM], F32, kind="ExternalOutput").ap()
    with tile.TileContext(nc) as tc:
        _emit(nc, tc, xT, us, vs, w, mask, out)
    nc.compile()
    return nc


def _tri_mask():
    # tri[rk, c] = 1.0 iff c >= rk  (keep where key index <= query index
    # within a diagonal 128x128 block)
    rk = np.arange(P)[:, None]
    c = np.arange(P)[None, :]
    return (c >= rk).astype(np.float32)


def _select_bank(U, V, logits, top_k):
    lg = np.asarray(logits, np.float32)
    e = np.exp(lg - lg.max())
    wsoft = (e / e.sum()).astype(np.float32)
    ti = np.argsort(-wsoft, kind="stable")[:top_k]
    tw = wsoft[ti]
    tw = tw / tw.sum()
    Ucat = np.concatenate([U[i] for i in ti], axis=1)          # [d, k*r]
    Vcat = np.concatenate([tw[k] * V[ti[k]] for k in range(top_k)], axis=0)
    return np.ascontiguousarray(Ucat, np.float32), np.ascontiguousarray(Vcat, np.float32)


def kernel(**inputs):
    x = np.asarray(inputs["x"], np.float32)          # [4, S, d]
    out_w = np.asarray(inputs["out_w"], np.float32)  # [d, d]
    top_k = int(np.asarray(inputs["top_k"]))
    assert top_k * 64 == KR, f"kernel compiled for top_k=4, got {top_k}"
    B = x.shape[0]

    cats = {}
    for b in "qkv":
        cats[b] = _select_bank(
            np.asarray(inputs[f"{b}_U"], np.float32),
            np.asarray(inputs[f"{b}_V"], np.float32),
            inputs[f"{b}_logits"], top_k)

    if "nc" not in _cache:
        _cache["nc"] = _build()
    nc = _cache["nc"]

    mask = _tri_mask()
    wT = np.ascontiguousarray(out_w.T, np.float32)   # [feat, d_model]
    in_maps = []
    for c in range(8):
        b, g = c // 2, c % 2
        m = {"xT": np.ascontiguousarray(x[b].T).astype(ml_dtypes.bfloat16),
             "mask": mask,
             "w": np.ascontiguousarray(wT[g * F:(g + 1) * F, :])}
        for bank in "qkv":
            Ucat, Vcat = cats[bank]
            m[f"u{bank}"] = Ucat.astype(ml_dtypes.bfloat16)
            m[f"v{bank}"] = np.ascontiguousarray(
                Vcat[:, g * F:(g + 1) * F]).astype(ml_dtypes.bfloat16)
        in_maps.append(m)

    res = run_bass_kernel_spmd(nc, in_maps, core_ids=list(range(8)), trace=TRACE)
    if TRACE:
        _cache["last_results"] = res
    parts = [r["out"] for r in res.results]
    full = np.stack([parts[2 * b] + parts[2 * b + 1] for b in range(B)])
    return full.astype(np.float32)
